# revision 3
# baseline (speedup 1.0000x reference)
"""KNN space regularizer kernel for Trainium2 (8 NeuronCores, SPMD).

Data-parallel over batch B=8: one batch element per core.  The axon
tunnel to the TRN2 host has ~60ms RTT and ~40MB/s effective bandwidth,
so wall time is transport-dominated; this kernel minimizes bytes and
buffer count shipped per call:

  - ONE packed uint8 input per core ([preds as fp16][x as fp32], so the
    indirect-gather source sits at offset 0), sharded over 8 cores.
  - no output placeholder operand (the bass_exec custom call allocates
    its result buffers).
  - fp16 output, upcast to fp32 on the host (~3e-4 rel rounding from
    fp16 preds+out vs the 2e-2 gate).

All preprocessing happens on-device (per core, N=4096 points, D=3):

  Bm [3,N] = x^T          (DMA transposed read)
  A  [3,N] = 2*x^T        (Act engine, exact in fp32)
  nsc[P,NT]= -|x_i|^2     (DVE square + adds in np.sum order)
  nsr[1,N] = nsc transposed to row layout (DRAM bounce)
  nsb[P,N] = nsr broadcast via K=1 ones-matmul (exact)

Per 128-row tile: s = (nsb + nsc_t) + 2<x_i,x_j> via PE fp32 matmul +
DVE scalar_tensor_tensor — the same values and operation order as the
on-device fp32 reference (both norm terms come from ONE computation,
so sub-ulp near-ties resolve identically; verified 0/32768 rows differ
from the jax reference beyond fp16 rounding).  Top-k (k =
argmax(k_vector)+1, computed on host like the torch .item()) selected
per row with DVE max_with_indices (+ match_replace round for k>8);
preds rows gathered from DRAM via per-row indirect DMA; mean in fp32,
written out as fp16.  sqrt/clamp of the reference are monotone so
ordering on -d2 matches ordering on the reference's distances.
"""

import sys

import numpy as np

sys.path.insert(0, "/opt/trn_rl_repo")
sys.path.insert(0, "/opt/trn_rl_repo/concourse")

N = 4096
D = 3
P = 128
NT = N // P  # 32 row tiles
HALF = 2048  # psum half width
MM = 512  # matmul free chunk (one PSUM bank)
NCORES = 8
PBYTES = N * D * 2  # fp16 preds bytes per core
XBYTES = N * D * 4  # fp32 x bytes per core
TOT = PBYTES + XBYTES

_CACHE = {}


def _build(k: int):
    import concourse.bass as bass
    import concourse.mybir as mybir
    import concourse.tile as tile
    from concourse import bacc

    f32 = mybir.dt.float32
    f16 = mybir.dt.float16
    u32 = mybir.dt.uint32
    u8 = mybir.dt.uint8
    nc = bacc.Bacc(
        "TRN2",
        target_bir_lowering=False,
        debug=False,
        num_devices=NCORES,
    )

    xb = nc.dram_tensor("xb", [TOT], u8, kind="ExternalInput").ap()
    out_d = nc.dram_tensor("out", [N, D], f16, kind="ExternalOutput").ap()
    # preds first: indirect_dma_start requires its source AP at offset 0
    ph_d = xb[0:PBYTES].bitcast(f16).rearrange("(n d) -> n d", d=D)
    x_d = xb[PBYTES:TOT].bitcast(f32).rearrange("(n d) -> n d", d=D)

    kk = min(k, 8)  # first-round take
    k2 = k - kk  # second-round take (k > 8)

    with tile.TileContext(nc) as tc:
        with (
            tc.tile_pool(name="const", bufs=1) as constp,
            tc.tile_pool(name="psum", bufs=2, space="PSUM") as psump,
            tc.tile_pool(name="sbig", bufs=2) as sp,
            tc.tile_pool(name="small", bufs=3) as smallp,
            tc.tile_pool(name="gath", bufs=2) as gp,
            tc.tile_pool(name="dscr", bufs=1, space="DRAM") as dp,
        ):
            # ---- on-device preprocessing ----
            Bm = constp.tile([3, N], f32)  # x^T
            nc.sync.dma_start(Bm[:], x_d.transpose([1, 0]))
            A = constp.tile([3, N], f32)  # 2*x^T (exact in fp32)
            nc.scalar.mul(A[:], Bm[:], 2.0)
            ones1 = constp.tile([1, P], f32)
            nc.gpsimd.memset(ones1[:], 1.0)

            # nsc [P, NT]: -|x_i|^2 with tile t in column t, row i=t*P+p
            # in partition p.  Square-reduce the 3 coords in the same
            # (x0^2+x1^2)+x2^2 order as np.sum; this single computation
            # feeds BOTH the row and the column term so sub-ulp near-ties
            # resolve exactly as in the fp32 reference.
            xt = constp.tile([P, NT, D], f32)
            nc.sync.dma_start(xt[:], x_d.rearrange("(t p) d -> p t d", p=P))
            xsq = constp.tile([P, NT, D], f32)
            nc.vector.tensor_mul(xsq[:], xt[:], xt[:])
            tmp = constp.tile([P, NT], f32)
            nc.vector.tensor_add(tmp[:], xsq[:, :, 0], xsq[:, :, 1])
            sqc = constp.tile([P, NT], f32)
            nc.vector.tensor_add(sqc[:], tmp[:], xsq[:, :, 2])
            nsc = constp.tile([P, NT], f32)
            nc.scalar.mul(nsc[:], sqc[:], -1.0)

            # nsr [1, N] = nsc transposed to row layout (DRAM bounce),
            # then nsb = broadcast to 128 partitions via K=1 ones-matmul
            # (1.0 * v + 0 is exact in fp32)
            scr = dp.tile([NT, P], f32)
            nc.sync.dma_start(scr[:].transpose([1, 0]), nsc[:])
            nsr = constp.tile([1, N], f32)
            nc.sync.dma_start(nsr[:], scr[:].rearrange("t p -> () (t p)"))
            nsb = constp.tile([P, N], f32)
            for h in range(2):
                ps2 = psump.tile([P, HALF], f32, tag="ps")
                for c in range(HALF // MM):
                    j0 = h * HALF + c * MM
                    nc.tensor.matmul(
                        ps2[:, c * MM : (c + 1) * MM],
                        ones1[:],
                        nsr[:, j0 : j0 + MM],
                        start=True,
                        stop=True,
                    )
                nc.scalar.copy(nsb[:, h * HALF : (h + 1) * HALF], ps2[:])

            mo = constp.tile([P, NT, D], f16)  # output accumulator
            for t in range(NT):
                s_sb = sp.tile([P, N], f32, tag="s_sb")
                for h in range(2):
                    ps = psump.tile([P, HALF], f32, tag="ps")
                    for c in range(HALF // MM):
                        j0 = h * HALF + c * MM
                        nc.tensor.matmul(
                            ps[:, c * MM : (c + 1) * MM],
                            A[:, t * P : (t + 1) * P],
                            Bm[:, j0 : j0 + MM],
                            start=True,
                            stop=True,
                        )
                    nc.vector.scalar_tensor_tensor(
                        out=s_sb[:, h * HALF : (h + 1) * HALF],
                        in0=nsb[:, h * HALF : (h + 1) * HALF],
                        scalar=nsc[:, t : t + 1],
                        in1=ps[:],
                        op0=mybir.AluOpType.add,
                        op1=mybir.AluOpType.add,
                    )

                val8 = smallp.tile([P, 8], f32, tag="val8")
                idx8 = smallp.tile([P, 8], u32, tag="idx8")
                nc.vector.max_with_indices(val8[:], idx8[:], s_sb[:])

                g = gp.tile([P, k, D], f16, tag="g")
                for r in range(kk):
                    nc.gpsimd.indirect_dma_start(
                        out=g[:, r, :],
                        out_offset=None,
                        in_=ph_d,
                        in_offset=bass.IndirectOffsetOnAxis(
                            ap=idx8[:, r : r + 1], axis=0
                        ),
                    )

                if k2 > 0:
                    s_mr = sp.tile([P, N], f32, tag="s_mr")
                    nc.vector.match_replace(
                        out=s_mr[:],
                        in_to_replace=val8[:],
                        in_values=s_sb[:],
                        imm_value=-1e30,
                    )
                    val8b = smallp.tile([P, 8], f32, tag="val8b")
                    idx8b = smallp.tile([P, 8], u32, tag="idx8b")
                    nc.vector.max_with_indices(val8b[:], idx8b[:], s_mr[:])
                    for r in range(k2):
                        nc.gpsimd.indirect_dma_start(
                            out=g[:, kk + r, :],
                            out_offset=None,
                            in_=ph_d,
                            in_offset=bass.IndirectOffsetOnAxis(
                                ap=idx8b[:, r : r + 1], axis=0
                            ),
                        )

                gf = smallp.tile([P, k * D], f32, tag="gf")
                nc.scalar.copy(gf[:], g[:].rearrange("p a b -> p (a b)"))
                acc = smallp.tile([P, D], f32, tag="acc")
                nc.vector.tensor_add(acc[:], gf[:, 0:D], gf[:, D : 2 * D])
                for r in range(2, k):
                    nc.vector.tensor_add(
                        acc[:], acc[:], gf[:, r * D : (r + 1) * D]
                    )
                nc.scalar.mul(mo[:, t, :], acc[:], 1.0 / k)

            nc.sync.dma_start(
                out_d.rearrange("(t p) d -> p t d", p=P), mo[:]
            )

    nc.compile()
    return nc


def _make_runner(nc):
    """Build the shard_map-jitted executor ONCE per compiled module.

    The jit module must be a pure bass_exec custom call (the neuronx-cc
    hook rejects any other op), with operands exactly the jit parameters
    in order.  Output placeholders are NOT passed: the custom call's
    result buffers serve as the outputs.
    """
    import jax
    from jax.experimental.shard_map import shard_map
    from jax.sharding import Mesh, PartitionSpec

    import concourse.mybir as mybir
    from concourse import bass2jax

    bass2jax.install_neuronx_cc_hook()
    assert nc.dbg_addr is None  # built with debug=False
    partition_name = (
        nc.partition_id_tensor.name if nc.partition_id_tensor else None
    )
    in_names, out_names, out_avals = [], [], []
    for alloc in nc.m.functions[0].allocations:
        if not isinstance(alloc, mybir.MemoryLocationSet):
            continue
        name = alloc.memorylocations[0].name
        if alloc.kind == "ExternalInput":
            if name != partition_name:
                in_names.append(name)
        elif alloc.kind == "ExternalOutput":
            out_names.append(name)
            shape = tuple(alloc.tensor_shape)
            dtype = mybir.dt.np(alloc.dtype)
            out_avals.append(jax.core.ShapedArray(shape, dtype))
    all_names = tuple(in_names) + (
        (partition_name,) if partition_name else ()
    )

    def _body(*args):
        operands = list(args)
        if partition_name is not None:
            operands.append(bass2jax.partition_id_tensor())
        outs = bass2jax._bass_exec_p.bind(
            *operands,
            out_avals=tuple(out_avals),
            in_names=all_names,
            out_names=tuple(out_names),
            lowering_input_output_aliases=(),
            sim_require_finite=True,
            sim_require_nnan=True,
            nc=nc,
        )
        return tuple(outs)

    devices = jax.devices()[:NCORES]
    mesh = Mesh(np.asarray(devices), ("core",))
    sharded = jax.jit(
        shard_map(
            _body,
            mesh=mesh,
            in_specs=(PartitionSpec("core"),) * len(in_names),
            out_specs=(PartitionSpec("core"),) * len(out_avals),
            check_rep=False,
        ),
        keep_unused=True,
    )
    param_names = list(in_names)

    def run(arrs_by_name):
        out_arrs = sharded(*[arrs_by_name[n] for n in param_names])
        return {
            name: np.asarray(out_arrs[i]) for i, name in enumerate(out_names)
        }

    return run


def _pack(x, preds):
    ph = preds.astype(np.float16)
    return np.concatenate(
        [
            ph.reshape(NCORES, -1).view(np.uint8),
            x.reshape(NCORES, -1).view(np.uint8),
        ],
        axis=1,
    ).reshape(-1)


def kernel(x, preds, k_vector):
    x = np.ascontiguousarray(np.asarray(x), dtype=np.float32)
    preds = np.asarray(preds)
    k_vector = np.asarray(k_vector)
    k = int(np.argmax(k_vector)) + 1
    B = x.shape[0]
    assert x.shape == (B, N, D) and preds.shape == (B, N, D)
    assert B == NCORES

    if k == 1:
        # top-1 is just the self point (distance 0); mean == preds row
        return np.ascontiguousarray(preds, dtype=np.float32)

    if k not in _CACHE:
        nc = _build(k)
        try:
            runner = _make_runner(nc)
        except Exception:
            runner = None
        _CACHE[k] = (nc, runner)
    nc, runner = _CACHE[k]

    packed = _pack(x, preds)

    results = None
    if runner is not None:
        try:
            results = runner({"xb": packed})
        except Exception:
            results = None
    if results is None:
        from concourse.bass_utils import run_bass_kernel_spmd

        in_maps = [
            {"xb": np.ascontiguousarray(packed[b * TOT : (b + 1) * TOT])}
            for b in range(B)
        ]
        res = run_bass_kernel_spmd(
            nc, in_maps, core_ids=list(range(NCORES))
        ).results
        out16 = np.stack([res[b]["out"] for b in range(B)], axis=0)
        return out16.astype(np.float32)

    return results["out"].astype(np.float32).reshape(B, N, D)


if __name__ == "__main__":
    rng = np.random.default_rng(0)
    x = rng.standard_normal((8, N, D), dtype=np.float32)
    p = rng.standard_normal((8, N, D), dtype=np.float32)
    kv = rng.standard_normal((16,), dtype=np.float32)
    o = kernel(x, p, kv)
    print(o.shape, o.dtype)
